# revision 9
# baseline (speedup 1.0000x reference)
"""Gated Transformer-XL (4-layer) Trainium2 Bass kernel, data-parallel over 8 cores.

Reference model (hardcoded shapes):
  L=4 layers, B=32 batch, T=128 seq, M=128 mem, D=1024, H=16 heads, HD=64,
  K = M+T = 256, MAXD = K+T = 384.

Strategy:
  - Data-parallel over batch: 4 batch elements per core, 8 cores, no collectives.
  - All activations live in feature-major "T-layout": [D on partitions (8 chunks
    of 128), tokens on free dim (4 batch x 128 tokens = 512 cols)].
  - All matmuls run in bf16 (fp32 PSUM accumulation): weights are host-cast to
    bf16 (halving HBM traffic); activation operands get bf16 shadows written
    directly by the producing DVE/ACT op. The residual stream, softmax chain
    and GRU combines stay fp32.
  - All big D x D weights are host-pre-transposed to [in_feature, out_feature]
    so y = x @ W.T becomes matmul(lhsT=W.T chunk, rhs=xT chunk) with N=512.
  - Per-head QKV projections (shared HD x HD weight across heads) use
    host-built block-diagonal [128,128] lhsT operating on head-pair chunks,
    which coincide exactly with the T-layout partition chunks.
  - LayerNorm stats via ones-vector PE matmuls (partition-dim reduction) and
    fp32 PE broadcast of the [1, tok] stats back to 128 partitions.
  - Relative-position attention term computed as BDr[q, m] = (qh+v) . rel_rev[m]
    with rel_rev[m] = rel_all[clip(255-m, 0, .)], then shifted per-row via a
    DRAM round-trip read with a diagonal access pattern
    (offset(q, k) = 127 + q*383 + k over a flat [128*384] buffer).
  - Softmax: row max via DVE (negated), exp+rowsum fused on ACT via accum_out,
    1/sum folded into the AV-output PSUM->SBUF copy.
  - GRU gates: both matmuls of each gate accumulate into the same PSUM banks;
    sigmoid/tanh applied directly from PSUM with fused per-partition bias.
  - SBUF tags are shared across lifetime-disjoint roles to fit 192KB/partition;
    PSUM runs as a single 8-slot ring of full banks.
"""

import numpy as np
import ml_dtypes

import concourse.bass as bass
import concourse.bacc as bacc
import concourse.tile as tile
from concourse import mybir
from concourse.alu_op_type import AluOpType
from concourse.masks import make_identity

F32 = mybir.dt.float32
BF16 = mybir.dt.bfloat16
AF = mybir.ActivationFunctionType
AX = mybir.AxisListType
NPBF = ml_dtypes.bfloat16

# model dims
L, B, T, M, D, H = 4, 32, 128, 128, 1024, 16
HD = D // H          # 64
K = M + T            # 256
MAXD = K + T         # 384
EPS = 1e-5
SCALE = float(np.float32(1.0) / np.float32(np.sqrt(np.float32(HD))))   # 0.125
NEG = np.float32(-1e20) * np.float32(SCALE)

N_CORES = 8
NB = B // N_CORES    # 4 batch elements per core
TOK = NB * T         # 512 query-token columns
JC = D // 128        # 8 feature chunks == head pairs
HP = H // 2          # 8 head pairs

# big-matmul weight order inside the packed w14 tensor
W14_ORDER = ["Wo",
             "ag_wr", "ag_ur", "ag_wz", "ag_uz", "ag_wg", "ag_ug",
             "Wfc",
             "fg_wr", "fg_ur", "fg_wz", "fg_uz", "fg_wg", "fg_ug"]
IW = {n: i for i, n in enumerate(W14_ORDER)}


def _pack_feat(vec):
    """[D] -> [128, JC] per-partition layout (feature f = jc*128 + p)."""
    return np.ascontiguousarray(vec.reshape(JC, 128).T)


def build_kernel(nb=NB, skip_v=True):
    """Build + compile the per-core Bass program."""
    tok = nb * T
    ktok = nb * M  # mem-group column count (== tok since M == T)
    nc = bacc.Bacc("TRN2", target_bir_lowering=False, debug=False,
                   num_devices=N_CORES)

    # ---- DRAM parameters (per-core) ----
    xT_d = nc.dram_tensor("xT", [JC, 128, tok], F32, kind="ExternalInput")
    memT_d = nc.dram_tensor("memT", [L, JC, 128, ktok], BF16, kind="ExternalInput")
    mbias_d = nc.dram_tensor("mbias", [nb, T, K], F32, kind="ExternalInput")
    posrev_d = nc.dram_tensor("posrev", [JC, 128, MAXD], BF16, kind="ExternalInput")
    w14_d = nc.dram_tensor("w14", [L, 14, JC, 128, D], BF16, kind="ExternalInput")
    wrT_d = nc.dram_tensor("wrT", [L, JC, 128, D], BF16, kind="ExternalInput")
    qkvB_d = nc.dram_tensor("qkvB", [L, 128, 3, 128], BF16, kind="ExternalInput")
    uv_d = nc.dram_tensor("uv", [L, 128, 2, HP], F32, kind="ExternalInput")
    lnw_d = nc.dram_tensor("lnw", [L, 128, 3, 2, JC], F32, kind="ExternalInput")
    gbias_d = nc.dram_tensor("gbias", [L, 128, 2, JC], F32, kind="ExternalInput")
    obias_d = nc.dram_tensor("obias", [L, 128, 2, JC], F32, kind="ExternalInput")
    out_d = nc.dram_tensor("out", [JC, 128, tok], F32, kind="ExternalOutput")

    # DRAM bounce ring for the BD diagonal shift (bf16)
    n_bounce = 8
    bdb = [nc.dram_tensor(f"bdb{i}", [128 * MAXD], BF16, kind="Internal")
           for i in range(n_bounce)]

    def diag_ap(d):
        ap = d.ap().copy()
        ap.ap = mybir.VecI64Pair([[MAXD - 1, 128], [1, K]])
        ap.offset = T - 1
        return ap

    with tile.TileContext(nc) as tc:
        with tc.tile_pool(name="cst", bufs=1) as cst, \
             tc.tile_pool(name="sb", bufs=1) as sb, \
             tc.tile_pool(name="ps", bufs=8, space="PSUM") as ps:

            def pstile(name):
                return ps.tile([128, 512], F32, tag="ps", bufs=8, name=name)

            ident = cst.tile([128, 128], F32)
            make_identity(nc, ident[:])
            identB = cst.tile([128, 128], BF16)
            nc.vector.tensor_copy(identB[:], ident[:])
            ones1 = cst.tile([128, 1], BF16)
            nc.vector.memset(ones1[:], 1.0)
            ones_row = cst.tile([1, 128], F32)
            nc.vector.memset(ones_row[:], 1.0)
            epst = cst.tile([1, 1], F32)
            nc.vector.memset(epst[:], EPS)
            mb = cst.tile([128, nb, K], F32)
            for b in range(nb):
                nc.sync.dma_start(mb[:, b, :], mbias_d[b])

            h = sb.tile([128, JC, tok], F32, tag="h", name="h0")
            for jc in range(JC):
                nc.sync.dma_start(h[:, jc, :], xT_d[jc])

            bounce_i = 0

            for l in range(L):
                # ---- per-layer small params ----
                lnw = sb.tile([128, 3, 2, JC], F32, tag="lnw", bufs=2, name=f"lnw{l}")
                nc.sync.dma_start(lnw[:], lnw_d[l])
                qkvB = sb.tile([128, 3, 128], BF16, tag="qkvB", bufs=2, name=f"qkvB{l}")
                nc.sync.dma_start(qkvB[:], qkvB_d[l])
                uv = sb.tile([128, 2, HP], F32, tag="uv", bufs=2, name=f"uv{l}")
                nc.sync.dma_start(uv[:], uv_d[l])
                gb = sb.tile([128, 2, JC], F32, tag="gb", bufs=2, name=f"gb{l}")
                nc.sync.dma_start(gb[:], gbias_d[l])
                ob = sb.tile([128, 2, JC], F32, tag="ob", bufs=2, name=f"ob{l}")
                nc.sync.dma_start(ob[:], obias_d[l])

                # bf16 shadow of the residual stream
                hb = sb.tile([128, JC, tok], BF16, tag="hb", name=f"hb{l}")
                for jc in range(JC):
                    nc.vector.tensor_copy(hb[:, jc, :], h[:, jc, :])

                # ---- rel_revT = Wr.T-projected reversed pos emb ----
                posrev = sb.tile([128, JC, MAXD], BF16, tag="G2", name=f"posrev{l}")
                for jc in range(JC):
                    nc.sync.dma_start(posrev[:, jc, :], posrev_d[jc])
                relrev = sb.tile([128, HP, MAXD], BF16, tag="G1", name=f"rel{l}")
                prls = [pstile(f"prl{l}_{t}") for t in range(8)]
                for jc in range(JC):
                    wc = sb.tile([128, D], BF16, tag="wchunk", bufs=2,
                                 name=f"wrc{l}_{jc}")
                    nc.sync.dma_start(wc[:], wrT_d[l, jc])
                    for t in range(8):
                        nc.tensor.matmul(prls[t][:, :MAXD],
                                         wc[:, t * 128:(t + 1) * 128],
                                         posrev[:, jc, :],
                                         start=(jc == 0), stop=(jc == JC - 1))
                for t in range(8):
                    nc.vector.tensor_copy(relrev[:, t, :], prls[t][:, :MAXD])

                # ---- LayerNorm + QKV projections, fused per feature-chunk ----
                khT = sb.tile([128, HP, 2, tok], BF16, tag="khT", name=f"khT{l}")
                vh = sb.tile([128, 2, nb, D], BF16, tag="vh", name=f"vh{l}")
                qhuT = sb.tile([128, HP, tok], BF16, tag="G3", name=f"qhuT{l}")
                if skip_v:
                    qhvT = qhuT
                else:
                    qhvT = sb.tile([128, HP, tok], BF16, tag="G5", name=f"qhvT{l}")

                def layer_norm_stats(src_fn, n, li):
                    """src_fn(jc) -> bf16 AP [128, n]. Returns (-mu, rstd) psums."""
                    pm = pstile(f"pm{li}")
                    pq = pstile(f"pq{li}")
                    for jc in range(JC):
                        src = src_fn(jc)
                        sqc = sb.tile([128, n], BF16, tag="sqc", bufs=2,
                                      name=f"sq{li}_{jc}")
                        nc.scalar.activation(sqc[:], src, AF.Square)
                        nc.tensor.matmul(pm[0:1, :n], ones1[:], src,
                                         start=(jc == 0), stop=(jc == JC - 1))
                        nc.tensor.matmul(pq[0:1, :n], ones1[:], sqc[:],
                                         start=(jc == 0), stop=(jc == JC - 1))
                    st = sb.tile([1, 3, n], F32, tag="stats", name=f"st{li}")
                    c0, c1, c2 = st[:, 0, :], st[:, 1, :], st[:, 2, :]
                    nc.scalar.activation(c0, pm[0:1, :n], AF.Copy, scale=-1.0 / D)
                    nc.scalar.activation(c1, pq[0:1, :n], AF.Copy, scale=1.0 / D)
                    nc.vector.tensor_tensor(c2, c0, c0, AluOpType.mult)
                    nc.vector.tensor_tensor(c1, c1, c2, AluOpType.subtract)
                    nc.scalar.activation(c2, c1, AF.Sqrt, bias=epst[:])
                    nc.vector.reciprocal(c1, c2)
                    pb = pstile(f"pb{li}")
                    pr = pstile(f"pr{li}")
                    nc.tensor.matmul(pb[:, :n], ones_row[:], c0, start=True,
                                     stop=True)
                    nc.tensor.matmul(pr[:, :n], ones_row[:], c1, start=True,
                                     stop=True)
                    return pb, pr

                def project_kv(kvg_jc, g, jc, width):
                    pk = pstile(f"pk{l}_{g}_{jc}")
                    nc.tensor.matmul(pk[:, :width], qkvB[:, 1, :], kvg_jc,
                                     start=True, stop=True)
                    nc.vector.tensor_copy(khT[:, jc, g, :width], pk[:, :width])
                    for b in range(nb):
                        pv = pstile(f"pv{l}_{g}_{jc}_{b}")
                        nc.tensor.matmul(pv[:, :128],
                                         kvg_jc[:, b * T:(b + 1) * T],
                                         qkvB[:, 2, :],
                                         start=True, stop=True)
                        nc.vector.tensor_copy(
                            vh[:, g, b, jc * 128:(jc + 1) * 128], pv[:, :128])

                # -- group 0: memory tokens (streamed from DRAM twice, bf16) --
                def src_mem(jc, tag_sfx):
                    c = sb.tile([128, ktok], BF16, tag="memc", bufs=2,
                                name=f"mem{tag_sfx}_{l}_{jc}")
                    nc.sync.dma_start(c[:], memT_d[l, jc])
                    return c[:]

                pb0, pr0 = layer_norm_stats(lambda jc: src_mem(jc, "s"), ktok,
                                            f"{l}m")
                for jc in range(JC):
                    tf = sb.tile([128, ktok], F32, tag="kvgf", bufs=2,
                                 name=f"tfm{l}_{jc}")
                    nc.vector.tensor_tensor(tf[:], src_mem(jc, "a"),
                                            pb0[:, :ktok], AluOpType.add)
                    nc.vector.tensor_tensor(tf[:], tf[:], pr0[:, :ktok],
                                            AluOpType.mult)
                    kvg = sb.tile([128, ktok], BF16, tag="kvg", bufs=2,
                                  name=f"kvm{l}_{jc}")
                    nc.vector.tensor_scalar(kvg[:], tf[:], lnw[:, 1, 0, jc:jc + 1],
                                            lnw[:, 1, 1, jc:jc + 1],
                                            AluOpType.mult, AluOpType.add)
                    project_kv(kvg[:], 0, jc, ktok)

                # -- group 1: current tokens (stats shared between ln_q/ln_kv) --
                pb1, pr1 = layer_norm_stats(lambda jc: hb[:, jc, :], tok, f"{l}h")
                for jc in range(JC):
                    tf = sb.tile([128, tok], F32, tag="kvgf", bufs=2,
                                 name=f"tfh{l}_{jc}")
                    nc.vector.tensor_tensor(tf[:], h[:, jc, :], pb1[:, :tok],
                                            AluOpType.add)
                    nc.vector.tensor_tensor(tf[:], tf[:], pr1[:, :tok],
                                            AluOpType.mult)
                    qng = sb.tile([128, tok], BF16, tag="qng", bufs=2,
                                  name=f"qn{l}_{jc}")
                    nc.vector.tensor_scalar(qng[:], tf[:], lnw[:, 0, 0, jc:jc + 1],
                                            lnw[:, 0, 1, jc:jc + 1],
                                            AluOpType.mult, AluOpType.add)
                    kvg = sb.tile([128, tok], BF16, tag="kvg", bufs=2,
                                  name=f"kvh{l}_{jc}")
                    nc.vector.tensor_scalar(kvg[:], tf[:], lnw[:, 1, 0, jc:jc + 1],
                                            lnw[:, 1, 1, jc:jc + 1],
                                            AluOpType.mult, AluOpType.add)
                    project_kv(kvg[:], 1, jc, tok)
                    pqh = pstile(f"pqh{l}_{jc}")
                    nc.tensor.matmul(pqh[:, :tok], qkvB[:, 0, :], qng[:],
                                     start=True, stop=True)
                    nc.vector.tensor_scalar_add(qhuT[:, jc, :], pqh[:, :tok],
                                                uv[:, 0, jc:jc + 1])
                    if not skip_v:
                        nc.vector.tensor_scalar_add(qhvT[:, jc, :], pqh[:, :tok],
                                                    uv[:, 1, jc:jc + 1])

                # ---- attention ----
                attoT = sb.tile([128, JC, tok], BF16, tag="G2", name=f"attoT{l}")
                for b in range(nb):
                    qs = slice(b * T, (b + 1) * T)
                    for hp in range(HP):
                        attc = sb.tile([128, 128], F32, tag="attc", bufs=2,
                                       name=f"attc{l}_{b}_{hp}")
                        for par in range(2):
                            hh = 2 * hp + par
                            es = slice(64 * par, 64 * par + 64)
                            u = f"{l}_{b}_{hh}"
                            pbd = pstile(f"pbd{u}")
                            nc.tensor.matmul(pbd[:, :MAXD], qhvT[es, hp, qs],
                                             relrev[es, hp, :],
                                             start=True, stop=True)
                            bd_sb = sb.tile([128, MAXD], BF16, tag="bd_sb", bufs=2,
                                            name=f"bds{u}")
                            nc.vector.tensor_scalar_mul(bd_sb[:], pbd[:, :MAXD],
                                                        SCALE)
                            dram = bdb[bounce_i % n_bounce]
                            bounce_i += 1
                            nc.sync.dma_start(
                                dram.ap().rearrange("(p f) -> p f", p=128),
                                bd_sb[:])
                            bdsh = sb.tile([128, K], BF16, tag="bdsh", bufs=2,
                                           name=f"bdh{u}")
                            nc.sync.dma_start(bdsh[:], diag_ap(dram))
                            pac = pstile(f"pac{u}")
                            nc.tensor.matmul(pac[:, :K], qhuT[es, hp, qs],
                                             khT[es, hp, :, qs],
                                             start=True, stop=True)
                            p_sb = sb.tile([128, K], F32, tag="p_sb", bufs=2,
                                           name=f"p{u}")
                            nc.vector.scalar_tensor_tensor(
                                p_sb[:], pac[:, :K], SCALE, mb[:, b, :],
                                AluOpType.mult, AluOpType.add)
                            nc.vector.tensor_tensor(p_sb[:], p_sb[:], bdsh[:],
                                                    AluOpType.add)
                            nmax = sb.tile([128, 1], F32, tag="nmax", bufs=2,
                                           name=f"nm{u}")
                            nc.vector.reduce_max(nmax[:], p_sb[:], axis=AX.X,
                                                 negate=True)
                            rsum = sb.tile([128, 1], F32, tag="rsum", bufs=2,
                                           name=f"rs{u}")
                            pexp = sb.tile([128, K], BF16, tag="pexp", bufs=2,
                                           name=f"pe{u}")
                            nc.scalar.activation(pexp[:], p_sb[:], AF.Exp,
                                                 bias=nmax[:], accum_out=rsum[:])
                            rinv = sb.tile([128, 1], F32, tag="rinv", bufs=2,
                                           name=f"ri{u}")
                            nc.vector.reciprocal(rinv[:], rsum[:])
                            attnT = sb.tile([128, 2, T], BF16, tag="attnT",
                                            bufs=2, name=f"aT{u}")
                            for kc in range(2):
                                ptr = ps.tile([128, 128], BF16, tag="ps",
                                              bufs=8, name=f"ptr{u}_{kc}")
                                nc.tensor.transpose(
                                    ptr[:],
                                    pexp[:, kc * 128:(kc + 1) * 128], identB[:])
                                nc.vector.tensor_copy(attnT[:, kc, :], ptr[:])
                            pav = pstile(f"pav{u}")
                            for kc in range(2):
                                nc.tensor.matmul(
                                    pav[:, :HD], attnT[:, kc, :],
                                    vh[:, kc, b, hh * HD:(hh + 1) * HD],
                                    start=(kc == 0), stop=(kc == 1))
                            nc.vector.tensor_scalar_mul(
                                attc[:, par * 64:(par + 1) * 64], pav[:, :HD],
                                rinv[:])
                        pat = pstile(f"pat{l}_{b}_{hp}")
                        nc.tensor.transpose(pat[:, :128], attc[:], ident[:])
                        nc.vector.tensor_copy(attoT[:, hp, qs], pat[:, :128])

                # ---- big matmuls (bf16 weights vs bf16 rhs) ----
                def bigmm(names, rhss, li):
                    pss = [pstile(f"pg{li}_{it}") for it in range(8)]
                    nmat = len(names)
                    for i, (nm, rhs) in enumerate(zip(names, rhss)):
                        for jc in range(JC):
                            wc = sb.tile([128, D], BF16, tag="wchunk", bufs=2,
                                         name=f"wc{li}_{nm}_{jc}")
                            nc.sync.dma_start(wc[:], w14_d[l, IW[nm], jc])
                            for it in range(8):
                                nc.tensor.matmul(
                                    pss[it][:, :tok],
                                    wc[:, it * 128:(it + 1) * 128],
                                    rhs[:, jc, :],
                                    start=(i == 0 and jc == 0),
                                    stop=(i == nmat - 1 and jc == JC - 1))
                    return pss

                def act_from_psum(pss, func, tag, li, bias=None, dtype=F32):
                    o = sb.tile([128, JC, tok], dtype, tag=tag, name=f"o{tag}{li}")
                    for it in range(8):
                        nc.scalar.activation(
                            o[:, it, :], pss[it][:, :tok], func,
                            bias=bias[:, it:it + 1] if bias is not None else 0.0)
                    return o

                def gru_gate(y_bf, x, x_bf, pfx, li, out_tag):
                    pss = bigmm([f"{pfx}_wr", f"{pfx}_ur"], [y_bf, x_bf], f"{li}r")
                    rx = sb.tile([128, JC, tok], BF16, tag="G2", name=f"rx{li}")
                    for it in range(8):
                        rc = sb.tile([128, tok], F32, tag="rc", bufs=2,
                                     name=f"rc{li}_{it}")
                        nc.scalar.activation(rc[:], pss[it][:, :tok], AF.Sigmoid)
                        nc.vector.tensor_tensor(rx[:, it, :], rc[:], x[:, it, :],
                                                AluOpType.mult)
                    gslot = 0 if pfx == "ag" else 1
                    z = act_from_psum(
                        bigmm([f"{pfx}_wz", f"{pfx}_uz"], [y_bf, x_bf], f"{li}z"),
                        AF.Sigmoid, "G3", f"{li}", bias=gb[:, gslot, :])
                    pss = bigmm([f"{pfx}_wg", f"{pfx}_ug"], [y_bf, rx], f"{li}g")
                    o = sb.tile([128, JC, tok], F32, tag=out_tag, name=f"go{li}")
                    for it in range(8):
                        hgc = sb.tile([128, tok], F32, tag="hgc", bufs=2,
                                      name=f"hgc{li}_{it}")
                        nc.scalar.activation(hgc[:], pss[it][:, :tok], AF.Tanh)
                        nc.vector.tensor_tensor(hgc[:], hgc[:], x[:, it, :],
                                                AluOpType.subtract)
                        nc.vector.tensor_tensor(hgc[:], hgc[:], z[:, it, :],
                                                AluOpType.mult)
                        nc.vector.tensor_tensor(o[:, it, :], hgc[:], x[:, it, :],
                                                AluOpType.add)
                    return o

                y = act_from_psum(bigmm(["Wo"], [attoT], f"{l}o"), AF.Relu,
                                  "G1", f"{l}y", bias=ob[:, 0, :], dtype=BF16)
                xg = gru_gate(y, h, hb, "ag", f"{l}a", "G5")
                xgb = sb.tile([128, JC, tok], BF16, tag="xgb", name=f"xgb{l}")
                for jc in range(JC):
                    nc.vector.tensor_copy(xgb[:, jc, :], xg[:, jc, :])

                # ---- FFN ----
                pb2, pr2 = layer_norm_stats(lambda jc: xgb[:, jc, :], tok, f"{l}f")
                fn = sb.tile([128, JC, tok], BF16, tag="G2", name=f"fn{l}")
                for jc in range(JC):
                    tf = sb.tile([128, tok], F32, tag="kvgf", bufs=2,
                                 name=f"tff{l}_{jc}")
                    nc.vector.tensor_tensor(tf[:], xg[:, jc, :], pb2[:, :tok],
                                            AluOpType.add)
                    nc.vector.tensor_tensor(tf[:], tf[:], pr2[:, :tok],
                                            AluOpType.mult)
                    nc.vector.tensor_scalar(fn[:, jc, :], tf[:],
                                            lnw[:, 2, 0, jc:jc + 1],
                                            lnw[:, 2, 1, jc:jc + 1],
                                            AluOpType.mult, AluOpType.add)
                fr = act_from_psum(bigmm(["Wfc"], [fn], f"{l}c"), AF.Relu,
                                   "G1", f"{l}f", bias=ob[:, 1, :], dtype=BF16)
                h = gru_gate(fr, xg, xgb, "fg", f"{l}f", "h")

            for jc in range(JC):
                nc.sync.dma_start(out_d[jc], h[:, jc, :])

    nc.compile()
    return nc


_NC_CACHE = {}


def _get_nc(nb=NB, skip_v=True):
    key = (nb, skip_v)
    if key not in _NC_CACHE:
        _NC_CACHE[key] = build_kernel(nb, skip_v)
    return _NC_CACHE[key]


def _pos_emb_rev():
    inv = 10000.0 ** (-np.arange(0, D, 2, dtype=np.float64) / D)
    seq = np.arange(MAXD, dtype=np.float64)[:, None] * inv[None, :]
    pe = np.concatenate([np.sin(seq), np.cos(seq)], axis=-1).astype(np.float32)
    rev_idx = np.maximum(0, (K - 1) - np.arange(MAXD))
    pe_rev = pe[rev_idx]                      # [MAXD, D]
    return np.ascontiguousarray(pe_rev.T.reshape(JC, 128, MAXD)).astype(NPBF)


def _block_diag(w):
    """[HD, HD] -> [128, 128] blockdiag(W.T, W.T)."""
    out = np.zeros((128, 128), np.float32)
    wt = np.ascontiguousarray(w.T)
    out[:HD, :HD] = wt
    out[HD:, HD:] = wt
    return out


def preprocess(x, memories, mask, params, nb=NB):
    """Build the shared (weights) and per-core (data) input maps."""
    def t_pack(mat):                      # [D, D] W -> W.T packed [JC, 128, D]
        return np.ascontiguousarray(mat.T.reshape(JC, 128, D))

    w14 = np.stack([
        np.stack([t_pack(np.asarray(params[n][l])) for n in W14_ORDER])
        for l in range(L)]).astype(NPBF)                      # [L,14,JC,128,D]
    wrT = np.stack([t_pack(np.asarray(params["Wr"][l]))
                    for l in range(L)]).astype(NPBF)

    qkvB = np.stack([
        np.stack([_block_diag(np.asarray(params[n][l]))
                  for n in ("Wq", "Wk", "Wv")], axis=1)
        for l in range(L)]).astype(NPBF)                      # [L,128,3,128]

    def uv_pack(vec):                     # [H, HD] -> [128, HP]
        o = np.zeros((128, HP), np.float32)
        for hh in range(H):
            o[64 * (hh % 2):64 * (hh % 2) + 64, hh // 2] = vec[hh]
        return o

    uv = np.stack([
        np.stack([uv_pack(np.asarray(params["u"][l])),
                  uv_pack(np.asarray(params["v"][l]))], axis=1)
        for l in range(L)])                                   # [L,128,2,HP]

    lnw = np.stack([
        np.stack([
            np.stack([_pack_feat(np.asarray(params[f"ln_{g}_s"][l])),
                      _pack_feat(np.asarray(params[f"ln_{g}_b"][l]))], axis=1)
            for g in ("q", "kv", "f")], axis=1)
        for l in range(L)])                                   # [L,128,3,2,JC]

    gbias = np.stack([
        np.stack([_pack_feat(-np.asarray(params["ag_bg"][l])),
                  _pack_feat(-np.asarray(params["fg_bg"][l]))], axis=1)
        for l in range(L)])                                   # [L,128,2,JC]
    obias = np.stack([
        np.stack([_pack_feat(np.asarray(params["bo"][l])),
                  _pack_feat(np.asarray(params["bfc"][l]))], axis=1)
        for l in range(L)])

    posrev = _pos_emb_rev()

    shared = {"w14": w14, "wrT": wrT, "qkvB": qkvB, "uv": uv, "lnw": lnw,
              "gbias": gbias, "obias": obias, "posrev": posrev}

    skip_v = bool(np.array_equal(np.asarray(params["u"]),
                                 np.asarray(params["v"])))

    n_cores = x.shape[0] // nb
    in_maps = []
    for c in range(n_cores):
        sl = slice(c * nb, (c + 1) * nb)
        xc = np.asarray(x[sl])            # [nb, T, D]
        xT = np.ascontiguousarray(
            xc.reshape(nb, T, JC, 128).transpose(2, 3, 0, 1)
        ).reshape(JC, 128, nb * T)
        mc = np.asarray(memories[:, sl])  # [L, nb, M, D]
        memT = np.ascontiguousarray(
            mc.reshape(L, nb, M, JC, 128).transpose(0, 3, 4, 1, 2)
        ).reshape(L, JC, 128, nb * M).astype(NPBF)
        mbias = np.where(np.asarray(mask[sl]), np.float32(0.0), NEG
                         ).astype(np.float32)
        in_maps.append({"xT": xT, "memT": memT, "mbias": mbias, **shared})
    return in_maps, skip_v


def postprocess(results, nb=NB):
    outs = []
    for r in results:
        o = r["out"].reshape(JC, 128, nb, T).transpose(2, 3, 0, 1)
        outs.append(o.reshape(nb, T, D))
    return np.ascontiguousarray(np.concatenate(outs, axis=0))


def kernel(x, memories, mask, params):
    from concourse.bass_utils import run_bass_kernel_spmd
    in_maps, skip_v = preprocess(x, memories, mask, params)
    nc = _get_nc(NB, skip_v)
    res = run_bass_kernel_spmd(nc, in_maps, core_ids=list(range(N_CORES)))
    return postprocess(res.results)
